# revision 1
# baseline (speedup 1.0000x reference)
"""Trainium2 Bass kernel for NoHiT: TR-product reconstruction + two 3x3 convs.

Sharding: S1 (the conv W dimension, index i) split across 8 cores, 128 output
columns each. Each core recomputes a 4-column halo of X locally from a sliced
Z1, so there are no collectives.

Per-core layout: column-groups of 4 i-columns x 32 channels on the 128 SBUF
partitions, j (S2) on the free dimension.
  X[i, j, k]  -> tiles Xg[(phase, k), 1 + j], groups of 4 columns
  conv = 3 banded [128x128] matmuls (dj taps; di folded into the band) plus
  corner 32x32 matmuls against neighbor groups via tile_position.
Evictions: PSUM -> SBUF with fused bias+LeakyReLU on the Scalar engine (Lrelu).
Matmuls run in float32r (full-rate fp32, ~1.5e-4 rel err on TRN2).

The final (S2, S1, S3) permutation happens during the host-side unshard: the
device writes its natural [(group, phase, oc), j] layout contiguously.
"""

import numpy as np

import concourse.bacc as bacc
import concourse.mybir as mybir
from concourse.tile import TileContext
from concourse.bass_utils import run_bass_kernel_spmd

S1, S2, S3 = 1024, 1024, 32
R = 16
NCORES = 8
NG = 34          # X / y1 column groups per core (1 halo group each side)
NGO = 32         # output column groups per core
JW = S2 + 2      # padded j width (zero col at each end)
F32 = mybir.dt.float32
F32R = mybir.dt.float32r

_CACHE = {}


def _build_nc():
    nc = bacc.Bacc("TRN2", target_bir_lowering=False)

    t0 = nc.dram_tensor("t0", [128, NG * 128], F32R, kind="ExternalInput")
    t1 = nc.dram_tensor("t1", [128, NG * 128], F32R, kind="ExternalInput")
    z0 = nc.dram_tensor("z0", [128, JW], F32R, kind="ExternalInput")
    z1 = nc.dram_tensor("z1", [128, JW], F32R, kind="ExternalInput")
    wb1 = nc.dram_tensor("wb1", [3, 128, 128], F32R, kind="ExternalInput")
    wb2 = nc.dram_tensor("wb2", [3, 128, 128], F32R, kind="ExternalInput")
    wsm1 = nc.dram_tensor("wsm1", [3, 128, 128], F32R, kind="ExternalInput")
    wsp1 = nc.dram_tensor("wsp1", [3, 128, 128], F32R, kind="ExternalInput")
    wsm2 = nc.dram_tensor("wsm2", [3, 128, 128], F32R, kind="ExternalInput")
    wsp2 = nc.dram_tensor("wsp2", [3, 128, 128], F32R, kind="ExternalInput")
    b1 = nc.dram_tensor("b1", [128, 1], F32, kind="ExternalInput")
    b2 = nc.dram_tensor("b2", [128, 1], F32, kind="ExternalInput")
    sl = nc.dram_tensor("sl", [128, 1], F32, kind="ExternalInput")
    blx = nc.dram_tensor("blx", [128, 1], F32, kind="ExternalInput")
    sr = nc.dram_tensor("sr", [128, 1], F32, kind="ExternalInput")
    brx = nc.dram_tensor("brx", [128, 1], F32, kind="ExternalInput")
    out = nc.dram_tensor("out", [NGO, 128, S2], F32, kind="ExternalOutput")

    LR = mybir.ActivationFunctionType.Lrelu

    with TileContext(nc) as tc:
        with tc.tile_pool(name="const", bufs=1) as const, \
             tc.tile_pool(name="xp", bufs=4) as xp, \
             tc.tile_pool(name="y1p", bufs=4) as y1p, \
             tc.tile_pool(name="y2p", bufs=3) as y2p, \
             tc.tile_pool(name="psx", bufs=2, space="PSUM") as psx, \
             tc.tile_pool(name="ps1", bufs=3, space="PSUM") as ps1, \
             tc.tile_pool(name="ps2", bufs=3, space="PSUM") as ps2:

            tt = [const.tile([128, NG * 128], F32R, tag="t0", name="tt0"),
                  const.tile([128, NG * 128], F32R, tag="t1", name="tt1")]
            nc.sync.dma_start(tt[0][:], t0[:])
            nc.sync.dma_start(tt[1][:], t1[:])
            zz = [const.tile([128, JW], F32R, tag="z0", name="zz0"),
                  const.tile([128, JW], F32R, tag="zz1", name="zz1")]
            nc.sync.dma_start(zz[0][:], z0[:])
            nc.sync.dma_start(zz[1][:], z1[:])

            wbt = {}
            wst = {}
            for cv, (wb, wsm, wsp) in ((1, (wb1, wsm1, wsp1)),
                                       (2, (wb2, wsm2, wsp2))):
                for t in range(3):
                    w = const.tile([128, 128], F32R, tag=f"wb{cv}_{t}", name=f"wb{cv}_{t}")
                    nc.sync.dma_start(w[:], wb[t, :, :])
                    wbt[(cv, t)] = w
                    sm = const.tile([128, 128], F32R, tag=f"wsm{cv}_{t}", name=f"wsm{cv}_{t}")
                    nc.sync.dma_start(sm[:], wsm[t, :, :])
                    sp = const.tile([128, 128], F32R, tag=f"wsp{cv}_{t}", name=f"wsp{cv}_{t}")
                    nc.sync.dma_start(sp[:], wsp[t, :, :])
                    wst[(cv, t, "m")] = sm
                    wst[(cv, t, "p")] = sp
            bt = {1: const.tile([128, 1], F32, tag="b1", name="bt1"),
                  2: const.tile([128, 1], F32, tag="b2", name="bt2")}
            nc.sync.dma_start(bt[1][:], b1[:])
            nc.sync.dma_start(bt[2][:], b2[:])
            edge = {}
            for nm, src in (("sl", sl), ("bl", blx), ("sr", sr), ("br", brx)):
                e_ = const.tile([128, 1], F32, tag=f"e{nm}", name=f"e{nm}")
                nc.sync.dma_start(e_[:], src[:])
                edge[nm] = e_
            zf = const.tile([128, 1], F32, tag="zf", name="zf")
            nc.gpsimd.memset(zf[:], 0.0)
            zc = const.tile([128, 1], F32R, tag="zc", name="zc")
            nc.vector.tensor_copy(zc[:], zf[:])

            xt = [None] * NG
            y1t = [None] * NG

            def conv(cv, src, dst_tiles, h, pspool, bias, last_is_fp32r):
                """Emit one conv output group h from src tiles into a new
                SBUF tile (returned)."""
                left = src[h - 1] if h - 1 >= 0 else None
                right = src[h + 1] if h + 1 < NG else None
                mid = src[h]
                odt = F32R if last_is_fp32r else F32
                ot = (y1p if cv == 1 else y2p).tile([128, JW if cv == 1 else S2],
                                                    odt, tag=f"y{cv}", name=f"y{cv}t")
                if cv == 1:
                    nc.vector.tensor_copy(ot[:, 0:1], zc[:])
                    nc.vector.tensor_copy(ot[:, JW - 1:JW], zc[:])
                steps = [(wbt[(cv, t)], mid, t) for t in range(3)]
                if left is not None:
                    steps += [(wst[(cv, t, "m")], left, t) for t in range(3)]
                if right is not None:
                    steps += [(wst[(cv, t, "p")], right, t) for t in range(3)]
                pss = [pspool.tile([128, 512], F32, tag=f"ps{cv}", name=f"ps{cv}c{ch}")
                       for ch in range(2)]
                nst = len(steps)
                for si, (w_, rhs_, t) in enumerate(steps):
                    for ch in range(2):
                        base = 1 + ch * 512
                        nc.tensor.matmul(
                            pss[ch][:], w_[:],
                            rhs_[:, base + t - 1: base + t - 1 + 512],
                            start=(si == 0), stop=(si == nst - 1))
                for ch in range(2):
                    ps = pss[ch]
                    dst_off = (1 if cv == 1 else 0) + ch * 512
                    if cv == 1 and h == 0:
                        sc, bi = edge["sl"][:], edge["bl"][:]
                    elif cv == 1 and h == NG - 1:
                        sc, bi = edge["sr"][:], edge["br"][:]
                    else:
                        sc, bi = 1.0, bias[:]
                    nc.scalar.activation(ot[:, dst_off: dst_off + 512], ps[:],
                                         LR, bias=bi, scale=sc, alpha=0.01)
                return ot

            for g in range(NG + 2):
                if g < NG:
                    # X-build for group g
                    x = xp.tile([128, JW], F32R, tag="x", name="xt_")
                    nc.vector.tensor_copy(x[:, 0:1], zc[:])
                    nc.vector.tensor_copy(x[:, JW - 1:JW], zc[:])
                    psxs = [psx.tile([128, 512], F32, tag="psx", name=f"psx{ch}")
                            for ch in range(2)]
                    for kr in range(2):
                        for ch in range(2):
                            nc.tensor.matmul(
                                psxs[ch][:], tt[kr][:, g * 128:(g + 1) * 128],
                                zz[kr][:, 1 + ch * 512: 1 + ch * 512 + 512],
                                start=(kr == 0), stop=(kr == 1))
                    for ch in range(2):
                        nc.vector.tensor_copy(x[:, 1 + ch * 512: 1 + ch * 512 + 512],
                                              psxs[ch][:])
                    xt[g] = x
                h = g - 1
                if 0 <= h < NG:
                    y1t[h] = conv(1, xt, y1t, h, ps1, bt[1], True)
                m = g - 2
                if 1 <= m <= NGO:
                    y2 = conv(2, y1t, None, m, ps2, bt[2], False)
                    nc.sync.dma_start(out[m - 1, :, :], y2[:])

    nc.finalize()
    return nc


def _host_prep(Z1, Z2, Z3, W1, b1, W2, b2):
    """Build per-core input maps (numpy layout prep only)."""
    # Z2v[(c,b), j], zero-padded j borders
    z2v = np.ascontiguousarray(Z2.transpose(2, 0, 1).reshape(256, S2))
    z2p = np.zeros((256, JW), np.float32)
    z2p[:, 1:1 + S2] = z2v

    def wblocks(W):
        wb = np.zeros((3, 128, 128), np.float32)
        for t in range(3):
            for pin in range(4):
                for pout in range(4):
                    d = pin - pout
                    if abs(d) <= 1:
                        wb[t, pin * 32:(pin + 1) * 32,
                           pout * 32:(pout + 1) * 32] = W[:, :, t, d + 1].T
        wsm = np.zeros((3, 128, 128), np.float32)
        wsp = np.zeros((3, 128, 128), np.float32)
        for t in range(3):
            wsm[t, 96:128, 0:32] = W[:, :, t, 0].T
            wsp[t, 0:32, 96:128] = W[:, :, t, 2].T
        return wb, wsm, wsp

    wb1, wsm1, wsp1 = wblocks(W1)
    wb2, wsm2, wsp2 = wblocks(W2)
    b1t = np.tile(b1, 4)[:, None].astype(np.float32)
    b2t = np.tile(b2, 4)[:, None].astype(np.float32)

    in_maps = []
    for c in range(NCORES):
        i0 = 128 * c - 4
        cols = NG * 4  # 136
        z1c = np.zeros((R, cols, R), np.float32)
        lo, hi = max(0, i0), min(S1, i0 + cols)
        z1c[:, lo - i0:hi - i0, :] = Z1[:, lo:hi, :]
        # T[cb, (i, k)] = sum_a Z3[c,k,a] Z1[a,i,b]
        t = np.einsum("cka,aib->cbik", Z3, z1c, optimize=True)
        t = np.ascontiguousarray(t.reshape(256, cols * 32)).astype(np.float32)
        ones = np.ones((128, 1), np.float32)
        zeros = np.zeros((128, 1), np.float32)
        in_maps.append({
            "sl": zeros if c == 0 else ones,
            "blx": zeros if c == 0 else b1t,
            "sr": zeros if c == NCORES - 1 else ones,
            "brx": zeros if c == NCORES - 1 else b1t,
            "t0": t[:128], "t1": t[128:],
            "z0": z2p[:128], "z1": z2p[128:],
            "wb1": wb1, "wb2": wb2,
            "wsm1": wsm1, "wsp1": wsp1, "wsm2": wsm2, "wsp2": wsp2,
            "b1": b1t, "b2": b2t,
        })
    return in_maps


def kernel(Z1, Z2, Z3, W1, b1, W2, b2, _trace=False, _trace_kwargs=None):
    Z1 = np.asarray(Z1, np.float32)
    Z2 = np.asarray(Z2, np.float32)
    Z3 = np.asarray(Z3, np.float32)
    W1 = np.asarray(W1, np.float32)
    W2 = np.asarray(W2, np.float32)
    b1 = np.asarray(b1, np.float32)
    b2 = np.asarray(b2, np.float32)

    if "nc" not in _CACHE:
        _CACHE["nc"] = _build_nc()
    nc = _CACHE["nc"]

    in_maps = _host_prep(Z1, Z2, Z3, W1, b1, W2, b2)
    kw = {}
    if _trace:
        kw = {"trace": True, "trace_kwargs": _trace_kwargs or {}}
    res = run_bass_kernel_spmd(nc, in_maps, list(range(NCORES)), **kw)
    _CACHE["last_results"] = res

    out = np.empty((S2, S1, S3), np.float32)
    for c in range(NCORES):
        arr = res.results[c]["out"]  # (32, 128, 1024)
        blk = arr.reshape(NGO, 4, S3, S2).transpose(3, 0, 1, 2)
        out[:, 128 * c:128 * c + 128, :] = blk.reshape(S2, 128, S3)
    return out



# revision 4
# speedup vs baseline: 1.0780x; 1.0780x over previous
"""Trainium2 Bass kernel for NoHiT: TR-product reconstruction + two 3x3 convs.

Sharding: S1 (the conv W dimension, index i) split across 8 cores, 128 output
columns each. Each core recomputes a 4-column halo of X locally from a sliced
Z1, so there are no collectives.

Per-core layout: column-groups of 4 i-columns x 32 channels on the 128 SBUF
partitions, j (S2) on the free dimension.
  X[i, j, k]  -> tiles Xg[(phase, k), 1 + j], groups of 4 columns
  conv = 3 banded [128x128] matmuls (dj taps; di folded into the band) plus
  3 combined-edge [64x128] matmuls against a packed edge tile that holds the
  right neighbor's column 0 (partitions 0:32) and the left neighbor's column
  3 (partitions 32:64), built with SBUF->SBUF DMA copies.
Evictions: PSUM -> SBUF with fused bias+LeakyReLU on the Scalar engine (Lrelu).
Matmuls run in float32r (full-rate fp32, ~1.5e-4 rel err on TRN2).

The final (S2, S1, S3) permutation happens during the host-side unshard: the
device writes its natural [(group, phase, oc), j] layout contiguously.
"""

import numpy as np

import concourse.bacc as bacc
import concourse.mybir as mybir
from concourse.tile import TileContext
from concourse.bass_utils import run_bass_kernel_spmd

S1, S2, S3 = 1024, 1024, 32
R = 16
NCORES = 8
NG = 34          # X / y1 column groups per core (1 halo group each side)
NGO = 32         # output column groups per core
JW = S2 + 2      # padded j width (zero col at each end)
F32 = mybir.dt.float32
F32R = mybir.dt.float32r

_CACHE = {}


def _build_nc():
    nc = bacc.Bacc("TRN2", target_bir_lowering=False)

    t0 = nc.dram_tensor("t0", [128, NG * 128], F32R, kind="ExternalInput")
    t1 = nc.dram_tensor("t1", [128, NG * 128], F32R, kind="ExternalInput")
    z0 = nc.dram_tensor("z0", [128, JW], F32R, kind="ExternalInput")
    z1 = nc.dram_tensor("z1", [128, JW], F32R, kind="ExternalInput")
    wb1 = nc.dram_tensor("wb1", [3, 128, 128], F32R, kind="ExternalInput")
    wb2 = nc.dram_tensor("wb2", [3, 128, 128], F32R, kind="ExternalInput")
    we1 = nc.dram_tensor("we1", [3, 64, 128], F32R, kind="ExternalInput")
    we2 = nc.dram_tensor("we2", [3, 64, 128], F32R, kind="ExternalInput")
    b1 = nc.dram_tensor("b1", [128, 1], F32, kind="ExternalInput")
    b2 = nc.dram_tensor("b2", [128, 1], F32, kind="ExternalInput")
    sl = nc.dram_tensor("sl", [128, 1], F32, kind="ExternalInput")
    blx = nc.dram_tensor("blx", [128, 1], F32, kind="ExternalInput")
    sr = nc.dram_tensor("sr", [128, 1], F32, kind="ExternalInput")
    brx = nc.dram_tensor("brx", [128, 1], F32, kind="ExternalInput")
    out = nc.dram_tensor("out", [NGO, 128, S2], F32, kind="ExternalOutput")

    LR = mybir.ActivationFunctionType.Lrelu

    with TileContext(nc) as tc:
        with tc.tile_pool(name="const", bufs=1) as const, \
             tc.tile_pool(name="xp", bufs=4) as xp, \
             tc.tile_pool(name="y1p", bufs=4) as y1p, \
             tc.tile_pool(name="y2p", bufs=3) as y2p, \
             tc.tile_pool(name="e1p", bufs=4) as e1p, \
             tc.tile_pool(name="e2p", bufs=4) as e2p, \
             tc.tile_pool(name="psx", bufs=2, space="PSUM") as psx, \
             tc.tile_pool(name="ps1", bufs=3, space="PSUM") as ps1, \
             tc.tile_pool(name="ps2", bufs=3, space="PSUM") as ps2:

        # --- constants -------------------------------------------------
            tt = [const.tile([128, NG * 128], F32R, tag="t0", name="tt0"),
                  const.tile([128, NG * 128], F32R, tag="t1", name="tt1")]
            nc.sync.dma_start(tt[0][:], t0[:])
            nc.sync.dma_start(tt[1][:], t1[:])
            zz = [const.tile([128, JW], F32R, tag="z0", name="zz0"),
                  const.tile([128, JW], F32R, tag="zz1", name="zz1")]
            nc.sync.dma_start(zz[0][:], z0[:])
            nc.sync.dma_start(zz[1][:], z1[:])

            wbt = {}
            wet = {}
            for cv, (wb, we) in ((1, (wb1, we1)), (2, (wb2, we2))):
                for t in range(3):
                    w = const.tile([128, 128], F32R, tag=f"wb{cv}_{t}", name=f"wb{cv}_{t}")
                    nc.sync.dma_start(w[:], wb[t, :, :])
                    wbt[(cv, t)] = w
                    e = const.tile([64, 128], F32R, tag=f"we{cv}_{t}", name=f"we{cv}_{t}")
                    nc.sync.dma_start(e[:], we[t, :, :])
                    wet[(cv, t)] = e
            bt = {1: const.tile([128, 1], F32, tag="b1", name="bt1"),
                  2: const.tile([128, 1], F32, tag="b2", name="bt2")}
            nc.sync.dma_start(bt[1][:], b1[:])
            nc.sync.dma_start(bt[2][:], b2[:])
            edge = {}
            for nm, src in (("sl", sl), ("bl", blx), ("sr", sr), ("br", brx)):
                e_ = const.tile([128, 1], F32, tag=f"e{nm}", name=f"e{nm}")
                nc.sync.dma_start(e_[:], src[:])
                edge[nm] = e_
            zf = const.tile([128, 1], F32, tag="zf", name="zf")
            nc.gpsimd.memset(zf[:], 0.0)
            zc = const.tile([128, 1], F32R, tag="zc", name="zc")
            nc.vector.tensor_copy(zc[:], zf[:])
            zrow = const.tile([64, JW], F32, tag="zrow", name="zrow")
            nc.gpsimd.memset(zrow[:], 0.0)

            xt = [None] * NG
            y1t = [None] * NG
            e1t = [None] * NG
            e2t = [None] * (NGO + 2)

            def conv(cv, src, et, h, pspool, bias):
                """Emit one conv output group h from src tiles + packed edge
                tile into a new SBUF tile (returned)."""
                mid = src[h]
                odt = F32R if cv == 1 else F32
                ot = (y1p if cv == 1 else y2p).tile([128, JW if cv == 1 else S2],
                                                    odt, tag=f"y{cv}", name=f"y{cv}t")
                if cv == 1:
                    nc.vector.tensor_copy(ot[:, 0:1], zc[:])
                    nc.vector.tensor_copy(ot[:, JW - 1:JW], zc[:])
                steps = [(wbt[(cv, t)], mid, t) for t in range(3)]
                steps += [(wet[(cv, t)], et, t) for t in range(3)]
                pss = [pspool.tile([128, 512], F32, tag=f"ps{cv}", name=f"ps{cv}c{ch}")
                       for ch in range(2)]
                nst = len(steps)
                for si, (w_, rhs_, t) in enumerate(steps):
                    for ch in range(2):
                        base = 1 + ch * 512
                        nc.tensor.matmul(
                            pss[ch][:], w_[:],
                            rhs_[:, base + t - 1: base + t - 1 + 512],
                            start=(si == 0), stop=(si == nst - 1))
                for ch in range(2):
                    ps = pss[ch]
                    dst_off = (1 if cv == 1 else 0) + ch * 512
                    if cv == 1 and h == 0:
                        sc, bi = edge["sl"][:], edge["bl"][:]
                    elif cv == 1 and h == NG - 1:
                        sc, bi = edge["sr"][:], edge["br"][:]
                    else:
                        sc, bi = 1.0, bias[:]
                    nc.scalar.activation(ot[:, dst_off: dst_off + 512], ps[:],
                                         LR, bias=bi, scale=sc, alpha=0.01)
                return ot

            for g in range(NG + 2):
                if g < NG:
                    # X-build for group g
                    x = xp.tile([128, JW], F32R, tag="x", name="xt_")
                    nc.vector.tensor_copy(x[:, 0:1], zc[:])
                    nc.vector.tensor_copy(x[:, JW - 1:JW], zc[:])
                    psxs = [psx.tile([128, 512], F32, tag="psx", name=f"psx{ch}")
                            for ch in range(2)]
                    for kr in range(2):
                        for ch in range(2):
                            nc.tensor.matmul(
                                psxs[ch][:], tt[kr][:, g * 128:(g + 1) * 128],
                                zz[kr][:, 1 + ch * 512: 1 + ch * 512 + 512],
                                start=(kr == 0), stop=(kr == 1))
                    for ch in range(2):
                        nc.vector.tensor_copy(x[:, 1 + ch * 512: 1 + ch * 512 + 512],
                                              psxs[ch][:])
                    xt[g] = x

                    # edge tiles for conv1 (built from x halo columns)
                    if g == 0:
                        e1t[0] = e1p.tile([64, JW], F32R, tag="e1", name="e1t")
                        nc.vector.tensor_copy(e1t[0][32:64, :], zrow[32:64, :])
                    if g + 1 < NG:
                        e1t[g + 1] = e1p.tile([64, JW], F32R, tag="e1", name="e1t")
                        if g + 1 == NG - 1:
                            nc.vector.tensor_copy(e1t[g + 1][0:32, :], zrow[0:32, :])
                    if g >= 1:
                        nc.sync.dma_start(e1t[g - 1][0:32, :], x[0:32, :])
                    if g + 1 < NG:
                        nc.sync.dma_start(e1t[g + 1][32:64, :], x[96:128, :])

                h = g - 1
                if 0 <= h < NG:
                    y1 = conv(1, xt, e1t[h], h, ps1, bt[1])
                    y1t[h] = y1
                    # edge tiles for conv2 (built from y1 halo columns)
                    if 1 <= g <= NGO:
                        e2t[g] = e2p.tile([64, JW], F32R, tag="e2", name="e2t")
                        nc.sync.dma_start(e2t[g][32:64, :], y1[96:128, :])
                    if 1 <= g - 2 <= NGO:
                        nc.sync.dma_start(e2t[g - 2][0:32, :], y1[0:32, :])

                m = g - 2
                if 1 <= m <= NGO:
                    y2 = conv(2, y1t, e2t[m], m, ps2, bt[2])
                    nc.sync.dma_start(out[m - 1, :, :], y2[:])

    nc.finalize()
    return nc


def _host_prep(Z1, Z2, Z3, W1, b1, W2, b2):
    """Build per-core input maps (numpy layout prep only)."""
    # Z2v[(c,b), j], zero-padded j borders
    z2v = np.ascontiguousarray(Z2.transpose(2, 0, 1).reshape(256, S2))
    z2p = np.zeros((256, JW), np.float32)
    z2p[:, 1:1 + S2] = z2v

    def wblocks(W):
        wb = np.zeros((3, 128, 128), np.float32)
        for t in range(3):
            for pin in range(4):
                for pout in range(4):
                    d = pin - pout
                    if abs(d) <= 1:
                        wb[t, pin * 32:(pin + 1) * 32,
                           pout * 32:(pout + 1) * 32] = W[:, :, t, d + 1].T
        # packed edge weight: rows 0:32 = right neighbor col0 (di=+1 into
        # pout 3), rows 32:64 = left neighbor col3 (di=-1 into pout 0)
        we = np.zeros((3, 64, 128), np.float32)
        for t in range(3):
            we[t, 0:32, 96:128] = W[:, :, t, 2].T
            we[t, 32:64, 0:32] = W[:, :, t, 0].T
        return wb, we

    wb1, we1 = wblocks(W1)
    wb2, we2 = wblocks(W2)
    b1t = np.tile(b1, 4)[:, None].astype(np.float32)
    b2t = np.tile(b2, 4)[:, None].astype(np.float32)

    in_maps = []
    for c in range(NCORES):
        i0 = 128 * c - 4
        cols = NG * 4  # 136
        z1c = np.zeros((R, cols, R), np.float32)
        lo, hi = max(0, i0), min(S1, i0 + cols)
        z1c[:, lo - i0:hi - i0, :] = Z1[:, lo:hi, :]
        # T[cb, (i, k)] = sum_a Z3[c,k,a] Z1[a,i,b]
        t = np.einsum("cka,aib->cbik", Z3, z1c, optimize=True)
        t = np.ascontiguousarray(t.reshape(256, cols * 32)).astype(np.float32)
        ones = np.ones((128, 1), np.float32)
        zeros = np.zeros((128, 1), np.float32)
        in_maps.append({
            "sl": zeros if c == 0 else ones,
            "blx": zeros if c == 0 else b1t,
            "sr": zeros if c == NCORES - 1 else ones,
            "brx": zeros if c == NCORES - 1 else b1t,
            "t0": t[:128], "t1": t[128:],
            "z0": z2p[:128], "z1": z2p[128:],
            "wb1": wb1, "wb2": wb2,
            "we1": we1, "we2": we2,
            "b1": b1t, "b2": b2t,
        })
    return in_maps


def kernel(Z1, Z2, Z3, W1, b1, W2, b2, _trace=False, _trace_kwargs=None):
    Z1 = np.asarray(Z1, np.float32)
    Z2 = np.asarray(Z2, np.float32)
    Z3 = np.asarray(Z3, np.float32)
    W1 = np.asarray(W1, np.float32)
    W2 = np.asarray(W2, np.float32)
    b1 = np.asarray(b1, np.float32)
    b2 = np.asarray(b2, np.float32)

    if "nc" not in _CACHE:
        _CACHE["nc"] = _build_nc()
    nc = _CACHE["nc"]

    in_maps = _host_prep(Z1, Z2, Z3, W1, b1, W2, b2)
    kw = {}
    if _trace:
        kw = {"trace": True, "trace_kwargs": _trace_kwargs or {}}
    res = run_bass_kernel_spmd(nc, in_maps, list(range(NCORES)), **kw)
    _CACHE["last_results"] = res

    out = np.empty((S2, S1, S3), np.float32)
    for c in range(NCORES):
        arr = res.results[c]["out"]  # (32, 128, 1024)
        blk = arr.reshape(NGO, 4, S3, S2).transpose(3, 0, 1, 2)
        out[:, 128 * c:128 * c + 128, :] = blk.reshape(S2, 128, S3)
    return out
